# revision 39
# baseline (speedup 1.0000x reference)
"""ConvolutionalFilterManifold Trainium2 kernel.

Reference: a tiny "manifold" MLP maps q[B,1,8,8] -> per-sample 3x3 conv
filters w[B,8,8,3,3] and biases b[B,8]; the heavy op is a per-sample
conv2d over x[B,8,512,512] (pad 1, stride 1) -> y[B,8,512,512].

Strategy (MODE=packed, default): manifold on host (tiny, exact); conv
on 8 NeuronCores with pure batch data-parallelism (4 samples/core).
The 2e-2 rel-err gate admits bf16 wire data, halving HBM traffic vs
f32 (the memory roofline for this problem): the host packs x into
bf16 slabs laid out EXACTLY as the kernel's SBUF tiles consume them,
and the kernel writes bf16 output slabs that the host unpacks/upcasts.

Conv mapping per core: flat group index j = s*37 + G over all 148
(sample, 14-output-row group) pairs. Stationary A[s,dx][(slot*8+ic),
(ro*8+oc)] = w[s, oc, ic, slot-ro, dx] (banded block Toeplitz, K=128
partitions = 16 input rows x 8 in-chans, M=112 = 14 out rows x 8
out-chans). Out-of-image rows and the width pad are zero DATA in the
packed x, so no stationary variants, memsets, or edge-column clipping
are needed; the 3 dx taps are 3 PSUM-accumulating matmuls per group
with rhs column offsets 0/1/2 into the 514-wide padded rows. Bias is
folded into the PSUM->SBUF evacuation (tensor_scalar_add on DVE /
activation-add on ACT, alternating), output tile bf16.

Super-chunks of 8 groups give 8.2KB/4KB per-partition DMA lines (one
fat packet per partition per transfer; packets fan out round-robin
over the 16 DMA engines). Small head chunks (2 groups) let compute
start as soon as ~350KB has landed; small tail chunks + split evacs +
per-group output DMAs on two queues shorten the final drain. Measured
~122us on 8 cores: tensor engine ~97% busy (444 matmuls x 512 cols @
2.4GHz + mostly-hidden LDWEIGHTS), DMA engines ~95% busy mid-stream —
simultaneously at the compute and memory rooflines for this mapping.

Notes: f32r stationaries would self-load (no LDWEIGHTS) but walrus
rejects mixed 32-bit/non-32-bit matmul inputs, so bf16 x forces bf16
weights. PSUM matmul output is capped at one 2KB bank (N<=512 f32),
which fixes the 3-matmuls-per-group floor.

Hardware constraint discovered empirically: every TPB instruction has
ONE sync-wait slot (bf16 matmuls get 2 via the LDW+MM split; 4-byte
self-loading matmuls get just 1). The emission order below keeps every
instruction's Tile-assigned wait count within its slots, and the
TileContext drain is patched to spread its per-proc waits over nops.
"""

import os
import re
import sys

sys.path.insert(0, "/opt/trn_rl_repo")

import numpy as np  # noqa: E402

import bass_rust  # noqa: E402
import concourse.bass as bass  # noqa: E402
import concourse.mybir as mybir  # noqa: E402
from concourse.bass_utils import run_bass_kernel_spmd  # noqa: E402
from concourse.tile import TileContext  # noqa: E402
from concourse.vector_clock import ScopedClock  # noqa: E402

B, IC, OC = 32, 8, 8
H = W = 512
NCORES = 8
SPC = B // NCORES  # samples per core
TOUT = 14  # output rows per group
TIN = 16  # input rows per group (TOUT + 2)
NGRP = 37  # 36 full groups + one 8-row group
M_PART = OC * TOUT  # 112 psum partitions
# (group-start, n-groups) chunks; 4 groups -> 4 PSUM banks, x2 bufs = 8
CHUNKS = [(g, min(4, NGRP - g)) for g in range(0, NGRP, 4)]

MODE = os.environ.get("CFM_MODE", "packed")  # packed | split | f32r | f32 | pair

# ---- packed mode geometry: flat group index j = s*37 + G over all 148
# per-core groups, super-chunks of 8 groups (first chunk 4 so PSUM tiles
# of 4 groups stay aligned to j%4 == 0 and compute can start early).
NGRP_S = 37  # groups per sample
NJ = SPC * NGRP_S  # 148 flat groups per core
# small first chunk so compute starts early; small last chunks so the
# final evacuation + output DMA tail is short
CHUNKS_J = (
    [(0, 2), (2, 2)]
    + [(4 + 8 * k, 8) for k in range(17)]
    + [(140, 4), (144, 2), (146, 2)]
)
WPK = W + 2  # padded row width inside a packed slab
# f32r stationaries would self-load (no LDWEIGHTS) but walrus rejects
# mixed 32-bit/non-32-bit matmul inputs, so bf16 x forces bf16 weights
PACKED_F32R = os.environ.get("CFM_PACKED_F32R", "0") == "1"

_ORIG_DRAIN = TileContext._drain_and_barrier


def _patched_drain_and_barrier(self, tick_clock, wait_clock):
    gc = tick_clock.global_clock
    vals = [int(v) for v in re.findall(r"-?\d+", repr(gc))]
    for i, v in enumerate(vals):
        if v > 0:
            sub = [0] * len(vals)
            sub[i] = v
            nop = self.nc.sync.nop(nofuse=True)
            wait_clock.add_sem_waits(
                nop.ins, ScopedClock({None: bass_rust.VectorClock(sub)})
            )
    self.nc.sync.drain()
    self.nc.all_engine_barrier()
    assert self.sems is not None
    popped = self.nc._tile_sem_poison_stack.pop()
    assert popped is self._sem_poison
    self.nc.clear_and_free_semaphores(list(self.sems.allocated().values()))
    self.nc.all_engine_barrier()


TileContext._drain_and_barrier = _patched_drain_and_barrier


def _legalize_waits(nc):
    """Every TPB instruction encodes at most ONE sync wait. Tile can
    attach several (multi-queue DMA producers, tile-granular WAR
    fan-ins). Hoist the excess onto same-engine InstNoOps inserted
    right before the instruction — the engine then blocks on the same
    sem set, just sequentially."""
    for fn in nc.m.functions:
        for bb in fn.blocks:
            out, changed = [], False
            for inst in bb.instructions:
                si = inst.sync_info
                if si is not None and len(si.on_wait) > 1:
                    waits = list(si.on_wait)
                    for w in waits[:-1]:
                        out.append(
                            mybir.InstNoOp(
                                name=nc.get_next_instruction_name(),
                                engine=inst.engine,
                                bass_nofuse=True,
                                sync_info=mybir.SyncInfo(on_wait=[w], on_update=[]),
                            )
                        )
                    inst.sync_info = mybir.SyncInfo(
                        on_wait=waits[-1:], on_update=list(si.on_update)
                    )
                    changed = True
                out.append(inst)
            if changed:
                bb.instructions = out


def _dram_ap(t, ap_list, offset):
    a = t[:].copy()
    a.ap = bass_rust.VecI64Pair(ap_list)
    a.offset = offset
    return a


def _manifold(q, wm1, bm1, wm2, bm2, wt, bt, wb, bb):
    m = np.einsum("bihw,cihw->bc", q, wm1) + bm1
    m = np.where(m > 0, m, np.float32(0.01) * m).astype(np.float32)
    m = m @ wm2.T + bm2
    m = np.where(m > 0, m, np.float32(0.01) * m).astype(np.float32)
    w = np.einsum("bc,cokl->bokl", m, wt) + bt[None, :, None, None]
    w = w.reshape(B, OC, IC, 3, 3).astype(np.float32)
    b = (m @ wb.T + bb).astype(np.float32)
    return w, b


def _build_stationaries(w):
    """w: [B, OC, IC, 3, 3] -> A: [B, 3 variants, 3 dx, 128, 112] f32.

    A[s, v, dx, ri*8+ic, ro*8+oc] = w[s, oc, ic, ri-ro, dx] for
    0 <= ri-ro <= 2 else 0. Variant 1 zeroes rows ri=0 (G=0, row -1);
    variant 2 zeroes rows ri>=9 (G=36, rows >=512)."""
    A = np.zeros((B, 3, 3, 128, M_PART), np.float32)
    ro = np.arange(TOUT)
    for dy in range(3):
        ri = ro + dy  # 14 values in [dy, dy+13]
        # block [ri*8 + ic, ro*8 + oc] = w[:, oc, ic, dy, dx]
        # use advanced indexing over (ro, ic, oc)
        blk = w[:, :, :, dy, :]  # [B, OC, IC, 3dx]
        for t in range(TOUT):
            A[:, 0, :, (t + dy) * 8 : (t + dy) * 8 + 8, t * 8 : t * 8 + 8] = (
                blk.transpose(0, 3, 2, 1)  # [B, dx, IC, OC]
            )
    A[:, 1] = A[:, 0]
    A[:, 1, :, 0:8, :] = 0.0
    A[:, 2] = A[:, 0]
    A[:, 2, :, 72:, :] = 0.0
    return A


def _build_stationaries_pair(w):
    """Row-pair variant: A[B, 3v, 3dx, 2parity, 64, 112] with
    A[s,v,dx,p, rq*8+ic, ro*8+oc] = w[s,oc,ic, 2rq+p-ro, dx] for
    0 <= 2rq+p-ro <= 2. Variant 1 zeroes input row index ri=2rq+p == 0;
    variant 2 zeroes ri >= 9."""
    A = np.zeros((B, 3, 3, 2, 64, M_PART), np.float32)
    for p in range(2):
        for rq in range(8):
            ri = 2 * rq + p
            for ro in range(TOUT):
                dy = ri - ro
                if 0 <= dy <= 2:
                    A[:, 0, :, p, rq * 8 : rq * 8 + 8, ro * 8 : ro * 8 + 8] = w[
                        :, :, :, dy, :
                    ].transpose(0, 3, 2, 1)
    A[:, 1] = A[:, 0]
    A[:, 1, :, 0, 0:8, :] = 0.0  # ri = 0
    A[:, 2] = A[:, 0]
    for p in range(2):
        for rq in range(8):
            if 2 * rq + p >= 9:
                A[:, 2, :, p, rq * 8 : rq * 8 + 8, :] = 0.0
    return A


def _pack_inputs_packed(x):
    """x: [B, IC, H, W] f32 -> per-core list of flat bf16 arrays.

    Per core the DRAM image is the concatenation over chunks (j0, ng) of
    blocks [128, ng*WPK]: block[slot*8+ic, g*WPK + 1 + col] =
    x[samp(j0+g), ic, 14*G(j0+g)+slot-1, col] with zeros outside the
    image. Each (chunk, partition) line is ng*WPK contiguous elements
    -> one fat DMA packet per partition per chunk."""
    import ml_dtypes

    xpad = np.zeros((B, IC, H + 2, WPK), ml_dtypes.bfloat16)
    xpad[:, :, 1 : 1 + H, 1 : 1 + W] = x
    rows = 14 * np.arange(NGRP_S)[:, None] + np.arange(TIN)[None, :]  # [37, 16]
    rows = np.minimum(rows, H + 1)  # group 36 tail rows -> last (zero) pad row
    g_all = xpad[:, :, rows, :]  # [B, IC, 37, 16, WPK]
    # -> [B, 37, 128(slot*8+ic), WPK]
    g_all = g_all.transpose(0, 2, 3, 1, 4).reshape(B, NGRP_S, 128, WPK)
    cores = []
    for k in range(NCORES):
        sl = g_all[k * SPC : (k + 1) * SPC].reshape(NJ, 128, WPK)
        blocks = [
            np.ascontiguousarray(sl[j0 : j0 + ng].transpose(1, 0, 2)).reshape(-1)
            for j0, ng in CHUNKS_J
        ]
        cores.append(np.concatenate(blocks))
    return cores


def _unpack_output_packed(yr_cores):
    """Per-core flat bf16 output (concat over chunks of [112, ng*W]
    blocks) -> y [B, OC, H, W] f32. Group j covers out rows
    14*G(j)..+13 of sample samp(j); rows >= 512 are dropped."""
    y = np.empty((B, OC, H, W), np.float32)
    for k, yr in enumerate(yr_cores):
        off = 0
        for j0, ng in CHUNKS_J:
            blk = np.asarray(yr[off : off + M_PART * ng * W]).astype(np.float32)
            # [112, ng, W] -> [ng, 14, OC, W]
            blk = blk.reshape(TOUT, OC, ng, W).transpose(2, 0, 1, 3)
            for g in range(ng):
                j = j0 + g
                s, G = divmod(j, NGRP_S)
                r0 = 14 * G
                nr = min(TOUT, H - r0)
                y[k * SPC + s, :, r0 : r0 + nr, :] = blk[g, :nr].transpose(1, 0, 2)
            off += M_PART * ng * W
    return y


def _build_consts_packed(w, b):
    """w: [n, OC, IC, 3, 3], b: [n, OC] for one core (n = SPC) ->
    (c_a bf16 [128, n*3*112], c_b f32 [112, n]).

    c_a[slot*8+ic, (s*3+dx)*112 + ro*8+oc] = w[s, oc, ic, slot-ro, dx]
    for 0 <= slot-ro <= 2, else 0 (out-of-image rows are zero *data* in
    the packed x, so no stationary variants are needed)."""
    import ml_dtypes

    n = w.shape[0]
    A = np.zeros((128, n, 3, M_PART), np.float32)
    for ro in range(TOUT):
        for dy in range(3):
            # [n, dx, IC, OC] block at partition rows (ro+dy)*8..+8
            A[(ro + dy) * 8 : (ro + dy) * 8 + 8, :, :, ro * 8 : ro * 8 + 8] = (
                w[:, :, :, dy, :].transpose(2, 0, 3, 1)  # [IC, n, dx, OC]
            )
    c_a = np.ascontiguousarray(A.reshape(128, n * 3 * M_PART))
    if not PACKED_F32R:
        c_a = c_a.astype(ml_dtypes.bfloat16)
    c_b = np.zeros((M_PART, n), np.float32)
    for s in range(n):
        c_b[:, s] = np.tile(b[s], TOUT)
    return c_a, c_b


def _emit_conv_packed(nc, tc, xin, yout, c_a, c_b):
    """Packed-layout conv over flat groups j = s*37 + G: per super-chunk
    one fat input DMA [128, ng*WPK] bf16, 3 bf16 tap matmuls per group
    accumulating in PSUM f32 (PSUM tiles of 4 groups = 4 banks), bias
    folded into the PSUM->SBUF evacuation (tensor_scalar_add on DVE /
    activation-add on ACT, alternating), one fat output DMA
    [112, ng*W] bf16 per super-chunk."""
    f32 = mybir.dt.float32
    bf16 = mybir.dt.bfloat16
    import contextlib

    with contextlib.ExitStack() as ctx:
        consts = ctx.enter_context(tc.tile_pool(name="consts", bufs=1))
        inp = ctx.enter_context(tc.tile_pool(name="inp", bufs=5))
        outp = ctx.enter_context(tc.tile_pool(name="outp", bufs=4))
        # 2-bank PSUM tiles (2 groups) x 4 bufs: evacuations are ~1.2us
        # instead of 2.35us and PSUM rotation never stalls the matmuls
        psum = ctx.enter_context(tc.tile_pool(name="psum", bufs=4, space="PSUM"))

        # Head latency: chunk0's input goes on the vector queue while the
        # sync queue loads consts; c_a is split so the first matmuls only
        # wait for sample 0's stationaries, not the whole table.
        adt = mybir.dt.float32r if PACKED_F32R else bf16
        ca0_sb = consts.tile([128, 3 * M_PART], adt)
        nc.sync.dma_start(
            out=ca0_sb[:], in_=_dram_ap(c_a, [[SPC * 3 * M_PART, 128], [1, 3 * M_PART]], 0)
        )
        car_sb = consts.tile([128, (SPC - 1) * 3 * M_PART], adt)
        cb_sb = consts.tile([M_PART, SPC], f32)

        def _late_consts():
            # rest of the stationaries + biases: needed only from the
            # first evacuation (~14us) / first s=1 group (~35us) on, so
            # they queue behind the first input chunks
            nc.sync.dma_start(
                out=car_sb[:],
                in_=_dram_ap(
                    c_a,
                    [[SPC * 3 * M_PART, 128], [1, (SPC - 1) * 3 * M_PART]],
                    3 * M_PART,
                ),
            )
            nc.sync.dma_start(out=cb_sb[:], in_=c_b[:])

        def ca_slice(s, dx):
            if s == 0:
                return ca0_sb[:, dx * M_PART : (dx + 1) * M_PART]
            return car_sb[:, ((s - 1) * 3 + dx) * M_PART : ((s - 1) * 3 + dx + 1) * M_PART]

        evac = [
            lambda out, in0, sc: nc.vector.tensor_scalar_add(
                out=out, in0=in0, scalar1=sc
            ),
            lambda out, in0, sc: nc.scalar.add(out=out, in_=in0, add=sc),
        ]
        ci = 0
        xoff = 0
        yoff = 0
        for ch_i, (j0, ng) in enumerate(CHUNKS_J):
            ti = inp.tile([128, 8, WPK], bf16, tag="ti")
            nc.sync.dma_start(
                out=ti[:, 0:ng, :],
                in_=_dram_ap(xin, [[ng * WPK, 128], [WPK, ng], [1, WPK]], xoff),
            )
            if ch_i == 0:
                _late_consts()
            last2 = ch_i >= len(CHUNKS_J) - 2  # 2-group tail chunks
            to = outp.tile([M_PART, 8, W], bf16, tag="to")
            for p0 in range(0, ng, 2):  # one 2-bank PSUM tile per 2 groups
                npg = min(2, ng - p0)
                ps = psum.tile([M_PART, 2 * W], f32)
                for g in range(npg):
                    j = j0 + p0 + g
                    s = j // NGRP_S
                    for dx in range(3):
                        nc.tensor.matmul(
                            ps[:, g * W : (g + 1) * W],
                            ca_slice(s, dx),
                            ti[:, p0 + g, dx : dx + W],
                            start=(dx == 0),
                            stop=(dx == 2),
                            skip_group_check=True,
                        )
                # evacuation runs: split where the sample (bias) changes;
                # tail chunks go per group so the final drain is short
                g = 0
                while g < npg:
                    s = (j0 + p0 + g) // NGRP_S
                    ge = g + 1
                    while (
                        not last2
                        and ge < npg
                        and (j0 + p0 + ge) // NGRP_S == s
                    ):
                        ge += 1
                    evac[ci % 2](
                        to[:, p0 + g : p0 + ge, :],
                        ps[:, g * W : ge * W],
                        cb_sb[:, s : s + 1],
                    )
                    ci += 1
                    g = ge
            if last2:
                # one out-DMA per group, alternating queues
                for g in range(ng):
                    (nc.gpsimd if g % 2 == 0 else nc.scalar).dma_start(
                        out=_dram_ap(
                            yout, [[ng * W, M_PART], [1, W]], yoff + g * W
                        ),
                        in_=to[:, g, :],
                    )
            else:
                nc.gpsimd.dma_start(
                    out=_dram_ap(yout, [[ng * W, M_PART], [W, ng], [1, W]], yoff),
                    in_=to[:, 0:ng, :],
                )
            xoff += 128 * ng * WPK
            yoff += M_PART * ng * W


if MODE == "pair":
    _A_COLS = SPC * 3 * 3 * 2 * M_PART  # 8064
else:
    _A_COLS = SPC * 3 * 3 * M_PART  # 4032
_ONES_OFF = _A_COLS
_BIAS_OFF = _A_COLS + W
_BVEC_OFF = _A_COLS  # f32r mode: per-partition bias vectors instead of ones/row
_CONST_COLS = _A_COLS + SPC if MODE == "f32r" else _BIAS_OFF + SPC * M_PART


def _build_consts(A_core, b_core):
    """Pack per-core consts into one [128, _CONST_COLS] f32 image."""
    C = np.zeros((128, _CONST_COLS), np.float32)
    if MODE == "pair":
        C[:64, :_A_COLS] = A_core.transpose(4, 0, 1, 2, 3, 5).reshape(64, _A_COLS)
    else:
        C[:, :_A_COLS] = A_core.transpose(3, 0, 1, 2, 4).reshape(128, _A_COLS)
    if MODE == "f32r":
        # bias as [112, SPC] per-partition column vectors (DVE adds them)
        for s in range(SPC):
            C[0:M_PART, _BVEC_OFF + s] = np.tile(b_core[s], TOUT)
    else:
        C[0, _ONES_OFF : _ONES_OFF + W] = 1.0
        bias_block = np.repeat(b_core[:, None, :], TOUT, axis=1)  # [SPC, 14, 8]
        C[0, _BIAS_OFF :] = bias_block.reshape(-1)
    return C


def _a_col(s, v, dx):
    return ((s * 3 + v) * 3 + dx) * M_PART


def _a_col_pair(s, v, dx, p):
    return ((((s * 3 + v) * 3 + dx) * 2) + p) * M_PART


_HW = H * W  # per-channel plane, elements
_SAMP = IC * _HW  # per-sample elements


def _emit_conv(nc, tc, xin, yout, c32, cbf):
    """Emit the per-core conv program.

    xin: DRAM [SPC, IC, H, W]; yout: DRAM [SPC, OC, H, W]
    c32: DRAM [128, _CONST_COLS] (f32r or f32 depending on mode)
    cbf: DRAM [128, 2*_A_COLS] bf16 (split mode only: A_hi | A_lo)
    """
    f32 = mybir.dt.float32
    bf16 = mybir.dt.bfloat16
    cdt = mybir.dt.float32r if MODE in ("split", "f32r") else f32
    xdt = cdt if MODE in ("f32r", "f32") else f32

    import contextlib

    with contextlib.ExitStack() as ctx:
        consts = ctx.enter_context(tc.tile_pool(name="consts", bufs=1))
        inp = ctx.enter_context(tc.tile_pool(name="inp", bufs=3))
        hlp = ctx.enter_context(tc.tile_pool(name="hlp", bufs=3))
        outp = ctx.enter_context(tc.tile_pool(name="outp", bufs=3))
        psum = ctx.enter_context(tc.tile_pool(name="psum", bufs=2, space="PSUM"))

        c32_sb = consts.tile([128, _CONST_COLS], cdt)
        nc.sync.dma_start(out=c32_sb[:], in_=c32[:])
        if MODE == "split":
            cbf_sb = consts.tile([128, 2 * _A_COLS], bf16)
            nc.sync.dma_start(out=cbf_sb[:], in_=cbf[:])

        ones_ap = c32_sb[0:1, _ONES_OFF : _ONES_OFF + W]

        for s in range(SPC):
            for G0, NG in CHUNKS:
                ti = inp.tile([128, 4, W], xdt, tag="ti")
                # ---- input DMAs, one per group (DMA APs max 3 dims)
                for g in range(NG):
                    G = G0 + g
                    if G == 0:
                        # zero the row(-1) slab: stale SBUF could hold NaNs
                        # and 0-weight x NaN still poisons the accumulation
                        nc.vector.memset(ti[0:32, 0:1, :], 0.0)
                        nc.sync.dma_start(
                            out=ti[8:128, 0:1, :],
                            in_=_dram_ap(xin, [[W, 15], [_HW, IC], [1, W]], s * _SAMP),
                        )
                    elif G == NGRP - 1:
                        nc.vector.memset(ti[64:128, g : g + 1, :], 0.0)
                        nc.sync.dma_start(
                            out=ti[0:72, g : g + 1, :],
                            in_=_dram_ap(
                                xin, [[W, 9], [_HW, IC], [1, W]], s * _SAMP + 503 * W
                            ),
                        )
                    else:
                        nc.sync.dma_start(
                            out=ti[:, g : g + 1, :],
                            in_=_dram_ap(
                                xin,
                                [[W, TIN], [_HW, IC], [1, W]],
                                s * _SAMP + (14 * G - 1) * W,
                            ),
                        )
                if MODE == "split":
                    th = hlp.tile([128, 4, W], bf16, tag="th")
                    tl = hlp.tile([128, 4, W], bf16, tag="tl")
                    for g in range(NG):
                        nc.vector.tensor_copy(
                            out=th[:, g : g + 1, :], in_=ti[:, g : g + 1, :]
                        )
                        nc.vector.tensor_sub(
                            out=tl[:, g : g + 1, :],
                            in0=ti[:, g : g + 1, :],
                            in1=th[:, g : g + 1, :],
                        )

                to = outp.tile([M_PART, 4 * W], f32, tag="to")
                ps = psum.tile([M_PART, 4 * W], f32)

                for g in range(NG):
                    G = G0 + g
                    v = 1 if G == 0 else (2 if G == NGRP - 1 else 0)
                    pcol = g * W
                    # bias pre-load: psum[:, :] = bias x ones  (start)
                    nc.tensor.matmul(
                        ps[:, pcol : pcol + W],
                        c32_sb[0:1, _BIAS_OFF + s * M_PART : _BIAS_OFF + (s + 1) * M_PART],
                        ones_ap,
                        start=True,
                        stop=False,
                        skip_group_check=True,
                    )
                    # dx taps: out cols [lo,hi) <- x cols [lo+dx-1, hi+dx-1)
                    taps = []
                    for dx in range(3):
                        lo = max(0, 1 - dx)
                        hi = W - max(0, dx - 1)
                        taps.append((dx, lo, hi))
                    if MODE == "split":
                        mm_ops = []
                        for dx, lo, hi in taps:
                            ah = cbf_sb[:, _a_col(s, v, dx) : _a_col(s, v, dx) + M_PART]
                            al = cbf_sb[
                                :,
                                _A_COLS + _a_col(s, v, dx) : _A_COLS
                                + _a_col(s, v, dx)
                                + M_PART,
                            ]
                            xh = th[:, g, lo + dx - 1 : hi + dx - 1]
                            xl = tl[:, g, lo + dx - 1 : hi + dx - 1]
                            mm_ops.append((ah, xh))
                            mm_ops.append((ah, xl))
                            mm_ops.append((al, xh))
                        for i, (a_ap, x_ap) in enumerate(mm_ops):
                            dx, lo, hi = taps[i // 3]
                            nc.tensor.matmul(
                                ps[:, pcol + lo : pcol + hi],
                                a_ap,
                                x_ap,
                                start=False,
                                stop=(i == len(mm_ops) - 1),
                                skip_group_check=True,
                            )
                    else:
                        for i, (dx, lo, hi) in enumerate(taps):
                            nc.tensor.matmul(
                                ps[:, pcol + lo : pcol + hi],
                                c32_sb[:, _a_col(s, v, dx) : _a_col(s, v, dx) + M_PART],
                                ti[:, g, lo + dx - 1 : hi + dx - 1],
                                start=False,
                                stop=(i == 2),
                                skip_group_check=True,
                            )

                # PSUM -> SBUF (single DVE copy per chunk), then DMA out
                nc.vector.tensor_copy(out=to[:, : NG * W], in_=ps[:, : NG * W])
                for g in range(NG):
                    G = G0 + g
                    if G == NGRP - 1:
                        nc.sync.dma_start(
                            out=_dram_ap(
                                yout,
                                [[W, 8], [_HW, OC], [1, W]],
                                s * OC * _HW + 504 * W,
                            ),
                            in_=to[0:64, g * W : (g + 1) * W],
                        )
                    else:
                        nc.sync.dma_start(
                            out=_dram_ap(
                                yout,
                                [[W, TOUT], [_HW, OC], [1, W]],
                                s * OC * _HW + 14 * G * W,
                            ),
                            in_=to[:, g * W : (g + 1) * W],
                        )


WP = W + 2  # host-padded row width (zero col at each edge; f32r needs even N)
_HWP = H * WP
_SAMP_P = IC * _HWP


def _emit_conv_pair(nc, tc, xin, yout, c32):
    """Row-pair layout: partition = (rq*8+ic) in [0,64), each partition's
    tile slice holds TWO consecutive (width-padded) image rows (~4KB
    contiguous DMA packets). 6 f32r tap matmuls (3 dx x 2 parity) + bias
    per group; x is host-padded to width 514 so every tap is N=512."""
    f32 = mybir.dt.float32
    f32r = mybir.dt.float32r
    import contextlib

    with contextlib.ExitStack() as ctx:
        consts = ctx.enter_context(tc.tile_pool(name="consts", bufs=1))
        inp = ctx.enter_context(tc.tile_pool(name="inp", bufs=4))
        outp = ctx.enter_context(tc.tile_pool(name="outp", bufs=3))
        psum = ctx.enter_context(tc.tile_pool(name="psum", bufs=2, space="PSUM"))

        c32_sb = consts.tile([128, _CONST_COLS], f32r)
        nc.sync.dma_start(out=c32_sb[:], in_=c32[:])
        ones_ap = c32_sb[0:1, _ONES_OFF : _ONES_OFF + W]

        for s in range(SPC):
            for G0, NG in CHUNKS:
                ti = inp.tile([64, 4, 2, WP], f32r, tag="ti")
                for g in range(NG):
                    G = G0 + g
                    if G == 0:
                        nc.vector.memset(ti[0:8, g, 0:1, :].bitcast(f32), 0.0)  # row -1
                        nc.sync.dma_start(
                            out=ti[0:8, g, 1:2, :],
                            in_=_dram_ap(xin, [[_HWP, IC], [1, WP]], s * _SAMP_P),
                        )
                        nc.sync.dma_start(
                            out=ti[8:64, g, :, :],
                            in_=_dram_ap(
                                xin,
                                [[2 * WP, 7], [_HWP, IC], [1, 2 * WP]],
                                s * _SAMP_P + WP,
                            ),
                        )
                    elif G == NGRP - 1:
                        nc.vector.memset(ti[32:64, g, :, :].bitcast(f32), 0.0)
                        nc.sync.dma_start(
                            out=ti[0:32, g, :, :],
                            in_=_dram_ap(
                                xin,
                                [[2 * WP, 4], [_HWP, IC], [1, 2 * WP]],
                                s * _SAMP_P + 503 * WP,
                            ),
                        )
                        nc.sync.dma_start(
                            out=ti[32:40, g, 0:1, :],
                            in_=_dram_ap(
                                xin, [[_HWP, IC], [1, WP]], s * _SAMP_P + 511 * WP
                            ),
                        )
                    else:
                        nc.sync.dma_start(
                            out=ti[:, g, :, :],
                            in_=_dram_ap(
                                xin,
                                [[2 * WP, 8], [_HWP, IC], [1, 2 * WP]],
                                s * _SAMP_P + (14 * G - 1) * WP,
                            ),
                        )

                to = outp.tile([M_PART, 4 * W], f32, tag="to")
                ps = psum.tile([M_PART, 4 * W], f32)

                for g in range(NG):
                    G = G0 + g
                    v = 1 if G == 0 else (2 if G == NGRP - 1 else 0)
                    pcol = g * W
                    nc.tensor.matmul(
                        ps[:, pcol : pcol + W],
                        c32_sb[0:1, _BIAS_OFF + s * M_PART : _BIAS_OFF + (s + 1) * M_PART],
                        ones_ap,
                        start=True,
                        stop=False,
                        skip_group_check=True,
                    )
                    for i, (dx, p) in enumerate(
                        [(dx, p) for dx in range(3) for p in range(2)]
                    ):
                        col = _a_col_pair(s, v, dx, p)
                        nc.tensor.matmul(
                            ps[:, pcol : pcol + W],
                            c32_sb[0:64, col : col + M_PART],
                            ti[:, g, p, dx : dx + W],
                            start=False,
                            stop=(i == 5),
                            skip_group_check=True,
                        )

                nc.vector.tensor_copy(out=to[:, : NG * W], in_=ps[:, : NG * W])
                for g in range(NG):
                    G = G0 + g
                    if G == NGRP - 1:
                        nc.gpsimd.dma_start(
                            out=_dram_ap(
                                yout, [[W, 8], [_HW, OC], [1, W]],
                                s * OC * _HW + 504 * W,
                            ),
                            in_=to[0:64, g * W : (g + 1) * W],
                        )
                    else:
                        nc.gpsimd.dma_start(
                            out=_dram_ap(
                                yout, [[W, TOUT], [_HW, OC], [1, W]],
                                s * OC * _HW + 14 * G * W,
                            ),
                            in_=to[:, g * W : (g + 1) * W],
                        )


def _emit_conv_f32r(nc, tc, xin, yout, c32):
    """K=128 (16 rows x 8 ic) layout, f32r taps: 3 matmuls per group,
    bias folded into the DVE PSUM->SBUF copy (tensor_scalar_add). x is
    host-padded to width 514 so every tap is N=512 (f32r needs even N)."""
    f32 = mybir.dt.float32
    f32r = mybir.dt.float32r
    import contextlib

    with contextlib.ExitStack() as ctx:
        consts = ctx.enter_context(tc.tile_pool(name="consts", bufs=1))
        inp = ctx.enter_context(tc.tile_pool(name="inp", bufs=6))
        outp = ctx.enter_context(tc.tile_pool(name="outp", bufs=4))
        psum = ctx.enter_context(tc.tile_pool(name="psum", bufs=2, space="PSUM"))

        c32_sb = consts.tile([128, _CONST_COLS], f32r)
        nc.sync.dma_start(out=c32_sb[:], in_=c32[:])
        bias_s = [
            c32_sb[0:M_PART, _BVEC_OFF + s : _BVEC_OFF + s + 1].bitcast(f32)
            for s in range(SPC)
        ]

        ci = 0  # alternate in-DMA issue between sync and scalar queues
        for s in range(SPC):
            for G0, NG in CHUNKS:
                ti = inp.tile([128, 4, WP], f32r, tag="ti")
                for g in range(NG):
                    G = G0 + g
                    eng = nc.sync if ci % 2 == 0 else nc.scalar
                    ci += 1
                    if G == 0:
                        nc.vector.memset(ti[0:32, g : g + 1, :].bitcast(f32), 0.0)
                        eng.dma_start(
                            out=ti[8:128, g : g + 1, :],
                            in_=_dram_ap(
                                xin, [[WP, 15], [_HWP, IC], [1, WP]], s * _SAMP_P
                            ),
                        )
                    elif G == NGRP - 1:
                        nc.vector.memset(ti[64:128, g : g + 1, :].bitcast(f32), 0.0)
                        eng.dma_start(
                            out=ti[0:72, g : g + 1, :],
                            in_=_dram_ap(
                                xin,
                                [[WP, 9], [_HWP, IC], [1, WP]],
                                s * _SAMP_P + 503 * WP,
                            ),
                        )
                    else:
                        eng.dma_start(
                            out=ti[:, g : g + 1, :],
                            in_=_dram_ap(
                                xin,
                                [[WP, TIN], [_HWP, IC], [1, WP]],
                                s * _SAMP_P + (14 * G - 1) * WP,
                            ),
                        )

                to = outp.tile([M_PART, 4 * W], f32, tag="to")
                ps = psum.tile([M_PART, 4 * W], f32)

                for g in range(NG):
                    G = G0 + g
                    v = 1 if G == 0 else (2 if G == NGRP - 1 else 0)
                    pcol = g * W
                    for dx in range(3):
                        col = _a_col(s, v, dx)
                        nc.tensor.matmul(
                            ps[:, pcol : pcol + W],
                            c32_sb[:, col : col + M_PART],
                            ti[:, g, dx : dx + W],
                            start=(dx == 0),
                            stop=(dx == 2),
                            skip_group_check=True,
                        )
                nc.vector.tensor_scalar_add(
                    out=to[:, : NG * W], in0=ps[:, : NG * W], scalar1=bias_s[s]
                )
                for g in range(NG):
                    G = G0 + g
                    pcol = g * W
                    if G == NGRP - 1:
                        nc.gpsimd.dma_start(
                            out=_dram_ap(
                                yout, [[W, 8], [_HW, OC], [1, W]],
                                s * OC * _HW + 504 * W,
                            ),
                            in_=to[0:64, pcol : pcol + W],
                        )
                    else:
                        nc.gpsimd.dma_start(
                            out=_dram_ap(
                                yout, [[W, TOUT], [_HW, OC], [1, W]],
                                s * OC * _HW + 14 * G * W,
                            ),
                            in_=to[:, pcol : pcol + W],
                        )


_NC_CACHE = {}


def _get_nc():
    if MODE in _NC_CACHE:
        return _NC_CACHE[MODE]
    f32 = mybir.dt.float32
    bf16 = mybir.dt.bfloat16
    nc = bass.Bass("TRN2", target_bir_lowering=False, debug=False, num_devices=NCORES)
    if MODE == "packed":
        xin = nc.declare_dram_parameter("x", [NJ * 128, WPK], bf16, isOutput=False)
        c_a = nc.declare_dram_parameter(
            "c_a",
            [128, SPC * 3 * M_PART],
            mybir.dt.float32r if PACKED_F32R else bf16,
            isOutput=False,
        )
        c_b = nc.declare_dram_parameter("c_b", [M_PART, SPC], f32, isOutput=False)
        yout = nc.declare_dram_parameter("y", [NJ * M_PART, W], bf16, isOutput=True)
        with TileContext(nc) as tc:
            _emit_conv_packed(nc, tc, xin, yout, c_a, c_b)
        _legalize_waits(nc)
        _NC_CACHE[MODE] = nc
        return nc
    cdt = mybir.dt.float32r if MODE in ("split", "f32r", "pair") else f32
    xdt = cdt if MODE in ("f32r", "f32", "pair") else f32
    xshape = [SPC, IC, H, WP] if MODE in ("pair", "f32r") else [SPC, IC, H, W]
    xin = nc.declare_dram_parameter("x", xshape, xdt, isOutput=False)
    c32 = nc.declare_dram_parameter("c32", [128, _CONST_COLS], cdt, isOutput=False)
    cbf = None
    if MODE == "split":
        cbf = nc.declare_dram_parameter(
            "cbf", [128, 2 * _A_COLS], mybir.dt.bfloat16, isOutput=False
        )
    yout = nc.declare_dram_parameter("y", [SPC, OC, H, W], f32, isOutput=True)
    with TileContext(nc) as tc:
        if MODE == "pair":
            _emit_conv_pair(nc, tc, xin, yout, c32)
        elif MODE == "f32r":
            _emit_conv_f32r(nc, tc, xin, yout, c32)
        else:
            _emit_conv(nc, tc, xin, yout, c32, cbf)
    _legalize_waits(nc)
    _NC_CACHE[MODE] = nc
    return nc


def _prepare(q, x, wm1, bm1, wm2, bm2, wt, bt, wb, bb):
    q, x = np.asarray(q, np.float32), np.asarray(x, np.float32)
    args = [np.asarray(a, np.float32) for a in (wm1, bm1, wm2, bm2, wt, bt, wb, bb)]
    w, b = _manifold(q, *args)
    if MODE == "packed":
        xr_cores = _pack_inputs_packed(x)
        in_maps = []
        for k in range(NCORES):
            sl = slice(k * SPC, (k + 1) * SPC)
            c_a, c_b = _build_consts_packed(w[sl], b[sl])
            in_maps.append(
                {"x": xr_cores[k].reshape(NJ * 128, WPK), "c_a": c_a, "c_b": c_b}
            )
        return _get_nc(), in_maps
    A = _build_stationaries_pair(w) if MODE == "pair" else _build_stationaries(w)

    if MODE in ("pair", "f32r"):
        xp = np.zeros((B, IC, H, WP), np.float32)
        xp[:, :, :, 1 : 1 + W] = x
        x = xp

    in_maps = []
    for k in range(NCORES):
        sl = slice(k * SPC, (k + 1) * SPC)
        C = _build_consts(A[sl], b[sl])
        im = {"x": np.ascontiguousarray(x[sl]), "c32": C}
        if MODE == "split":
            import ml_dtypes

            Ablock = C[:, :_A_COLS].astype(np.float32)
            Ah = Ablock.astype(ml_dtypes.bfloat16)
            Al = (Ablock - Ah.astype(np.float32)).astype(ml_dtypes.bfloat16)
            im["cbf"] = np.concatenate([Ah, Al], axis=1)
        in_maps.append(im)
    return _get_nc(), in_maps


def kernel(q, x, wm1, bm1, wm2, bm2, wt, bt, wb, bb):
    nc, in_maps = _prepare(q, x, wm1, bm1, wm2, bm2, wt, bt, wb, bb)
    br = run_bass_kernel_spmd(nc, in_maps, list(range(NCORES)))
    if MODE == "packed":
        return _unpack_output_packed(
            [np.asarray(r["y"]).reshape(-1) for r in br.results]
        )
    return np.concatenate([r["y"] for r in br.results], axis=0)



# revision 42
# speedup vs baseline: 1.0971x; 1.0971x over previous
"""ConvolutionalFilterManifold Trainium2 kernel.

Reference: a tiny "manifold" MLP maps q[B,1,8,8] -> per-sample 3x3 conv
filters w[B,8,8,3,3] and biases b[B,8]; the heavy op is a per-sample
conv2d over x[B,8,512,512] (pad 1, stride 1) -> y[B,8,512,512].

Strategy (MODE=packed, default): manifold on host (tiny, exact); conv
on 8 NeuronCores with pure batch data-parallelism (4 samples/core).
The 2e-2 rel-err gate admits bf16 wire data, halving HBM traffic vs
f32 (the memory roofline for this problem): the host packs x into
bf16 slabs laid out EXACTLY as the kernel's SBUF tiles consume them,
and the kernel writes bf16 output slabs that the host unpacks/upcasts.

Conv mapping per core: flat group index j = s*37 + G over all 148
(sample, 14-output-row group) pairs. Stationary A[s,dx][(slot*8+ic),
(ro*8+oc)] = w[s, oc, ic, slot-ro, dx] (banded block Toeplitz, K=128
partitions = 16 input rows x 8 in-chans, M=112 = 14 out rows x 8
out-chans). Out-of-image rows and the width pad are zero DATA in the
packed x, so no stationary variants, memsets, or edge-column clipping
are needed; the 3 dx taps are 3 PSUM-accumulating matmuls per group
with rhs column offsets 0/1/2 into the 514-wide padded rows. Bias is
folded into the PSUM->SBUF evacuation (tensor_scalar_add on DVE /
activation-add on ACT, alternating), output tile bf16.

Super-chunks of 8 groups give 8.2KB/4KB per-partition DMA lines (one
fat packet per partition per transfer; packets fan out round-robin
over the 16 DMA engines). Small head chunks (2 groups) let compute
start as soon as ~350KB has landed; small tail chunks + split evacs +
per-group output DMAs on two queues shorten the final drain. Measured
~122us on 8 cores: tensor engine ~97% busy (444 matmuls x 512 cols @
2.4GHz + mostly-hidden LDWEIGHTS), DMA engines ~95% busy mid-stream —
simultaneously at the compute and memory rooflines for this mapping.

Notes: f32r stationaries would self-load (no LDWEIGHTS) but walrus
rejects mixed 32-bit/non-32-bit matmul inputs, so bf16 x forces bf16
weights. PSUM matmul output is capped at one 2KB bank (N<=512 f32),
which fixes the 3-matmuls-per-group floor.

Hardware constraint discovered empirically: every TPB instruction has
ONE sync-wait slot (bf16 matmuls get 2 via the LDW+MM split; 4-byte
self-loading matmuls get just 1). The emission order below keeps every
instruction's Tile-assigned wait count within its slots, and the
TileContext drain is patched to spread its per-proc waits over nops.
"""

import os
import re
import sys

sys.path.insert(0, "/opt/trn_rl_repo")

import numpy as np  # noqa: E402

import bass_rust  # noqa: E402
import concourse.bass as bass  # noqa: E402
import concourse.mybir as mybir  # noqa: E402
from concourse.bass_utils import run_bass_kernel_spmd  # noqa: E402
from concourse.tile import TileContext  # noqa: E402
from concourse.vector_clock import ScopedClock  # noqa: E402

B, IC, OC = 32, 8, 8
H = W = 512
NCORES = 8
SPC = B // NCORES  # samples per core
TOUT = 14  # output rows per group
TIN = 16  # input rows per group (TOUT + 2)
NGRP = 37  # 36 full groups + one 8-row group
M_PART = OC * TOUT  # 112 psum partitions
# (group-start, n-groups) chunks; 4 groups -> 4 PSUM banks, x2 bufs = 8
CHUNKS = [(g, min(4, NGRP - g)) for g in range(0, NGRP, 4)]

MODE = os.environ.get("CFM_MODE", "packed")  # packed | split | f32r | f32 | pair

# ---- packed mode geometry: flat group index j = s*37 + G over all 148
# per-core groups, super-chunks of 8 groups (first chunk 4 so PSUM tiles
# of 4 groups stay aligned to j%4 == 0 and compute can start early).
NGRP_S = 37  # groups per sample
NJ = SPC * NGRP_S  # 148 flat groups per core
# small first chunk so compute starts early; small last chunks so the
# final evacuation + output DMA tail is short
CHUNKS_J = (
    [(0, 2), (2, 2)]
    + [(4 + 8 * k, 8) for k in range(17)]
    + [(140, 4), (144, 2), (146, 2)]
)
WPK = W + 2  # padded row width inside a packed slab
# f32r stationaries would self-load (no LDWEIGHTS) but walrus rejects
# mixed 32-bit/non-32-bit matmul inputs, so bf16 x forces bf16 weights
PACKED_F32R = os.environ.get("CFM_PACKED_F32R", "0") == "1"

_ORIG_DRAIN = TileContext._drain_and_barrier


def _patched_drain_and_barrier(self, tick_clock, wait_clock):
    gc = tick_clock.global_clock
    vals = [int(v) for v in re.findall(r"-?\d+", repr(gc))]
    for i, v in enumerate(vals):
        if v > 0:
            sub = [0] * len(vals)
            sub[i] = v
            nop = self.nc.sync.nop(nofuse=True)
            wait_clock.add_sem_waits(
                nop.ins, ScopedClock({None: bass_rust.VectorClock(sub)})
            )
    self.nc.sync.drain()
    self.nc.all_engine_barrier()
    assert self.sems is not None
    popped = self.nc._tile_sem_poison_stack.pop()
    assert popped is self._sem_poison
    self.nc.clear_and_free_semaphores(list(self.sems.allocated().values()))
    self.nc.all_engine_barrier()


TileContext._drain_and_barrier = _patched_drain_and_barrier


def _legalize_waits(nc):
    """Every TPB instruction encodes at most ONE sync wait. Tile can
    attach several (multi-queue DMA producers, tile-granular WAR
    fan-ins). Hoist the excess onto same-engine InstNoOps inserted
    right before the instruction — the engine then blocks on the same
    sem set, just sequentially."""
    for fn in nc.m.functions:
        for bb in fn.blocks:
            out, changed = [], False
            for inst in bb.instructions:
                si = inst.sync_info
                if si is not None and len(si.on_wait) > 1:
                    waits = list(si.on_wait)
                    for w in waits[:-1]:
                        out.append(
                            mybir.InstNoOp(
                                name=nc.get_next_instruction_name(),
                                engine=inst.engine,
                                bass_nofuse=True,
                                sync_info=mybir.SyncInfo(on_wait=[w], on_update=[]),
                            )
                        )
                    inst.sync_info = mybir.SyncInfo(
                        on_wait=waits[-1:], on_update=list(si.on_update)
                    )
                    changed = True
                out.append(inst)
            if changed:
                bb.instructions = out


def _dram_ap(t, ap_list, offset):
    a = t[:].copy()
    a.ap = bass_rust.VecI64Pair(ap_list)
    a.offset = offset
    return a


def _manifold(q, wm1, bm1, wm2, bm2, wt, bt, wb, bb):
    m = np.einsum("bihw,cihw->bc", q, wm1) + bm1
    m = np.where(m > 0, m, np.float32(0.01) * m).astype(np.float32)
    m = m @ wm2.T + bm2
    m = np.where(m > 0, m, np.float32(0.01) * m).astype(np.float32)
    w = np.einsum("bc,cokl->bokl", m, wt) + bt[None, :, None, None]
    w = w.reshape(B, OC, IC, 3, 3).astype(np.float32)
    b = (m @ wb.T + bb).astype(np.float32)
    return w, b


def _build_stationaries(w):
    """w: [B, OC, IC, 3, 3] -> A: [B, 3 variants, 3 dx, 128, 112] f32.

    A[s, v, dx, ri*8+ic, ro*8+oc] = w[s, oc, ic, ri-ro, dx] for
    0 <= ri-ro <= 2 else 0. Variant 1 zeroes rows ri=0 (G=0, row -1);
    variant 2 zeroes rows ri>=9 (G=36, rows >=512)."""
    A = np.zeros((B, 3, 3, 128, M_PART), np.float32)
    ro = np.arange(TOUT)
    for dy in range(3):
        ri = ro + dy  # 14 values in [dy, dy+13]
        # block [ri*8 + ic, ro*8 + oc] = w[:, oc, ic, dy, dx]
        # use advanced indexing over (ro, ic, oc)
        blk = w[:, :, :, dy, :]  # [B, OC, IC, 3dx]
        for t in range(TOUT):
            A[:, 0, :, (t + dy) * 8 : (t + dy) * 8 + 8, t * 8 : t * 8 + 8] = (
                blk.transpose(0, 3, 2, 1)  # [B, dx, IC, OC]
            )
    A[:, 1] = A[:, 0]
    A[:, 1, :, 0:8, :] = 0.0
    A[:, 2] = A[:, 0]
    A[:, 2, :, 72:, :] = 0.0
    return A


def _build_stationaries_pair(w):
    """Row-pair variant: A[B, 3v, 3dx, 2parity, 64, 112] with
    A[s,v,dx,p, rq*8+ic, ro*8+oc] = w[s,oc,ic, 2rq+p-ro, dx] for
    0 <= 2rq+p-ro <= 2. Variant 1 zeroes input row index ri=2rq+p == 0;
    variant 2 zeroes ri >= 9."""
    A = np.zeros((B, 3, 3, 2, 64, M_PART), np.float32)
    for p in range(2):
        for rq in range(8):
            ri = 2 * rq + p
            for ro in range(TOUT):
                dy = ri - ro
                if 0 <= dy <= 2:
                    A[:, 0, :, p, rq * 8 : rq * 8 + 8, ro * 8 : ro * 8 + 8] = w[
                        :, :, :, dy, :
                    ].transpose(0, 3, 2, 1)
    A[:, 1] = A[:, 0]
    A[:, 1, :, 0, 0:8, :] = 0.0  # ri = 0
    A[:, 2] = A[:, 0]
    for p in range(2):
        for rq in range(8):
            if 2 * rq + p >= 9:
                A[:, 2, :, p, rq * 8 : rq * 8 + 8, :] = 0.0
    return A


def _pack_inputs_packed(x):
    """x: [B, IC, H, W] f32 -> per-core list of flat bf16 arrays.

    Per core the DRAM image is the concatenation over chunks (j0, ng) of
    blocks [128, ng*WPK]: block[slot*8+ic, g*WPK + 1 + col] =
    x[samp(j0+g), ic, 14*G(j0+g)+slot-1, col] with zeros outside the
    image. Each (chunk, partition) line is ng*WPK contiguous elements
    -> one fat DMA packet per partition per chunk."""
    import ml_dtypes

    xpad = np.zeros((B, IC, H + 2, WPK), ml_dtypes.bfloat16)
    xpad[:, :, 1 : 1 + H, 1 : 1 + W] = x
    rows = 14 * np.arange(NGRP_S)[:, None] + np.arange(TIN)[None, :]  # [37, 16]
    rows = np.minimum(rows, H + 1)  # group 36 tail rows -> last (zero) pad row
    g_all = xpad[:, :, rows, :]  # [B, IC, 37, 16, WPK]
    # -> [B, 37, 128(slot*8+ic), WPK]
    g_all = g_all.transpose(0, 2, 3, 1, 4).reshape(B, NGRP_S, 128, WPK)
    cores = []
    for k in range(NCORES):
        sl = g_all[k * SPC : (k + 1) * SPC].reshape(NJ, 128, WPK)
        blocks = [
            np.ascontiguousarray(sl[j0 : j0 + ng].transpose(1, 0, 2)).reshape(-1)
            for j0, ng in CHUNKS_J
        ]
        cores.append(np.concatenate(blocks))
    return cores


def _unpack_output_packed(yr_cores):
    """Per-core flat bf16 output (concat over chunks of [112, ng*W]
    blocks) -> y [B, OC, H, W] f32. Group j covers out rows
    14*G(j)..+13 of sample samp(j); rows >= 512 are dropped."""
    y = np.empty((B, OC, H, W), np.float32)
    for k, yr in enumerate(yr_cores):
        off = 0
        for j0, ng in CHUNKS_J:
            blk = np.asarray(yr[off : off + M_PART * ng * W]).astype(np.float32)
            # [112, ng, W] -> [ng, 14, OC, W]
            blk = blk.reshape(TOUT, OC, ng, W).transpose(2, 0, 1, 3)
            for g in range(ng):
                j = j0 + g
                s, G = divmod(j, NGRP_S)
                r0 = 14 * G
                nr = min(TOUT, H - r0)
                y[k * SPC + s, :, r0 : r0 + nr, :] = blk[g, :nr].transpose(1, 0, 2)
            off += M_PART * ng * W
    return y


def _build_consts_packed(w, b):
    """w: [n, OC, IC, 3, 3], b: [n, OC] for one core (n = SPC) ->
    (c_a bf16 [128, n*3*112], c_b f32 [112, n]).

    c_a[slot*8+ic, (s*3+dx)*112 + ro*8+oc] = w[s, oc, ic, slot-ro, dx]
    for 0 <= slot-ro <= 2, else 0 (out-of-image rows are zero *data* in
    the packed x, so no stationary variants are needed)."""
    import ml_dtypes

    n = w.shape[0]
    A = np.zeros((128, n, 3, M_PART), np.float32)
    for ro in range(TOUT):
        for dy in range(3):
            # [n, dx, IC, OC] block at partition rows (ro+dy)*8..+8
            A[(ro + dy) * 8 : (ro + dy) * 8 + 8, :, :, ro * 8 : ro * 8 + 8] = (
                w[:, :, :, dy, :].transpose(2, 0, 3, 1)  # [IC, n, dx, OC]
            )
    c_a = np.ascontiguousarray(A.reshape(128, n * 3 * M_PART))
    if not PACKED_F32R:
        c_a = c_a.astype(ml_dtypes.bfloat16)
    c_b = np.zeros((M_PART, n), np.float32)
    for s in range(n):
        c_b[:, s] = np.tile(b[s], TOUT)
    return c_a, c_b


def _emit_conv_packed(nc, tc, xin, yout, c_a, c_b):
    """Packed-layout conv over flat groups j = s*37 + G: per super-chunk
    one fat input DMA [128, ng*WPK] bf16, 3 bf16 tap matmuls per group
    accumulating in PSUM f32 (PSUM tiles of 4 groups = 4 banks), bias
    folded into the PSUM->SBUF evacuation (tensor_scalar_add on DVE /
    activation-add on ACT, alternating), one fat output DMA
    [112, ng*W] bf16 per super-chunk."""
    f32 = mybir.dt.float32
    bf16 = mybir.dt.bfloat16
    import contextlib

    with contextlib.ExitStack() as ctx:
        consts = ctx.enter_context(tc.tile_pool(name="consts", bufs=1))
        inp = ctx.enter_context(tc.tile_pool(name="inp", bufs=5))
        outp = ctx.enter_context(tc.tile_pool(name="outp", bufs=4))
        psum = ctx.enter_context(tc.tile_pool(name="psum", bufs=2, space="PSUM"))

        # Head latency: chunk0's input goes on the vector queue while the
        # sync queue loads consts; c_a is split so the first matmuls only
        # wait for sample 0's stationaries, not the whole table.
        adt = mybir.dt.float32r if PACKED_F32R else bf16
        ca0_sb = consts.tile([128, 3 * M_PART], adt)
        nc.sync.dma_start(
            out=ca0_sb[:], in_=_dram_ap(c_a, [[SPC * 3 * M_PART, 128], [1, 3 * M_PART]], 0)
        )
        car_sb = consts.tile([128, (SPC - 1) * 3 * M_PART], adt)
        cb_sb = consts.tile([M_PART, SPC], f32)

        def _late_consts():
            # rest of the stationaries + biases: needed only from the
            # first evacuation (~14us) / first s=1 group (~35us) on, so
            # they queue behind the first input chunks
            nc.sync.dma_start(
                out=car_sb[:],
                in_=_dram_ap(
                    c_a,
                    [[SPC * 3 * M_PART, 128], [1, (SPC - 1) * 3 * M_PART]],
                    3 * M_PART,
                ),
            )
            nc.sync.dma_start(out=cb_sb[:], in_=c_b[:])

        def ca_slice(s, dx):
            if s == 0:
                return ca0_sb[:, dx * M_PART : (dx + 1) * M_PART]
            return car_sb[:, ((s - 1) * 3 + dx) * M_PART : ((s - 1) * 3 + dx + 1) * M_PART]

        evac = [
            lambda out, in0, sc: nc.vector.tensor_scalar_add(
                out=out, in0=in0, scalar1=sc
            ),
            lambda out, in0, sc: nc.scalar.add(out=out, in_=in0, add=sc),
        ]
        ci = 0
        xoff = 0
        yoff = 0
        for ch_i, (j0, ng) in enumerate(CHUNKS_J):
            ti = inp.tile([128, 8, WPK], bf16, tag="ti")
            nc.sync.dma_start(
                out=ti[:, 0:ng, :],
                in_=_dram_ap(xin, [[ng * WPK, 128], [WPK, ng], [1, WPK]], xoff),
            )
            if ch_i == 0:
                _late_consts()
            last2 = ch_i >= len(CHUNKS_J) - 2  # 2-group tail chunks
            # near the tail, split evacs in two so PSUM buffers recycle
            # fast (a 2.35us 4-group evac otherwise stalls the last MMs)
            evac_fine = ch_i >= len(CHUNKS_J) - 3
            to = outp.tile([M_PART, 8, W], bf16, tag="to")
            for p0 in range(0, ng, 4):  # one PSUM tile per 4 groups
                npg = min(4, ng - p0)
                ps = psum.tile([M_PART, 4 * W], f32)
                for g in range(npg):
                    j = j0 + p0 + g
                    s = j // NGRP_S
                    for dx in range(3):
                        nc.tensor.matmul(
                            ps[:, g * W : (g + 1) * W],
                            ca_slice(s, dx),
                            ti[:, p0 + g, dx : dx + W],
                            start=(dx == 0),
                            stop=(dx == 2),
                            skip_group_check=True,
                        )
                # evacuation runs: split where the sample (bias) changes;
                # tail chunks split per group across both engines so the
                # final evac + out-DMA drain is short
                g = 0
                while g < npg:
                    s = (j0 + p0 + g) // NGRP_S
                    ge = g + 1
                    while (
                        not evac_fine
                        and ge < npg
                        and (j0 + p0 + ge) // NGRP_S == s
                    ):
                        ge += 1
                    if evac_fine and not last2:
                        ge = min(g + 2, npg)  # 2-group evac halves
                    evac[ci % 2](
                        to[:, p0 + g : p0 + ge, :],
                        ps[:, g * W : ge * W],
                        cb_sb[:, s : s + 1],
                    )
                    ci += 1
                    g = ge
            if last2:
                # one out-DMA per group, alternating queues
                for g in range(ng):
                    (nc.gpsimd if g % 2 == 0 else nc.scalar).dma_start(
                        out=_dram_ap(
                            yout, [[ng * W, M_PART], [1, W]], yoff + g * W
                        ),
                        in_=to[:, g, :],
                    )
            else:
                nc.gpsimd.dma_start(
                    out=_dram_ap(yout, [[ng * W, M_PART], [W, ng], [1, W]], yoff),
                    in_=to[:, 0:ng, :],
                )
            xoff += 128 * ng * WPK
            yoff += M_PART * ng * W


if MODE == "pair":
    _A_COLS = SPC * 3 * 3 * 2 * M_PART  # 8064
else:
    _A_COLS = SPC * 3 * 3 * M_PART  # 4032
_ONES_OFF = _A_COLS
_BIAS_OFF = _A_COLS + W
_BVEC_OFF = _A_COLS  # f32r mode: per-partition bias vectors instead of ones/row
_CONST_COLS = _A_COLS + SPC if MODE == "f32r" else _BIAS_OFF + SPC * M_PART


def _build_consts(A_core, b_core):
    """Pack per-core consts into one [128, _CONST_COLS] f32 image."""
    C = np.zeros((128, _CONST_COLS), np.float32)
    if MODE == "pair":
        C[:64, :_A_COLS] = A_core.transpose(4, 0, 1, 2, 3, 5).reshape(64, _A_COLS)
    else:
        C[:, :_A_COLS] = A_core.transpose(3, 0, 1, 2, 4).reshape(128, _A_COLS)
    if MODE == "f32r":
        # bias as [112, SPC] per-partition column vectors (DVE adds them)
        for s in range(SPC):
            C[0:M_PART, _BVEC_OFF + s] = np.tile(b_core[s], TOUT)
    else:
        C[0, _ONES_OFF : _ONES_OFF + W] = 1.0
        bias_block = np.repeat(b_core[:, None, :], TOUT, axis=1)  # [SPC, 14, 8]
        C[0, _BIAS_OFF :] = bias_block.reshape(-1)
    return C


def _a_col(s, v, dx):
    return ((s * 3 + v) * 3 + dx) * M_PART


def _a_col_pair(s, v, dx, p):
    return ((((s * 3 + v) * 3 + dx) * 2) + p) * M_PART


_HW = H * W  # per-channel plane, elements
_SAMP = IC * _HW  # per-sample elements


def _emit_conv(nc, tc, xin, yout, c32, cbf):
    """Emit the per-core conv program.

    xin: DRAM [SPC, IC, H, W]; yout: DRAM [SPC, OC, H, W]
    c32: DRAM [128, _CONST_COLS] (f32r or f32 depending on mode)
    cbf: DRAM [128, 2*_A_COLS] bf16 (split mode only: A_hi | A_lo)
    """
    f32 = mybir.dt.float32
    bf16 = mybir.dt.bfloat16
    cdt = mybir.dt.float32r if MODE in ("split", "f32r") else f32
    xdt = cdt if MODE in ("f32r", "f32") else f32

    import contextlib

    with contextlib.ExitStack() as ctx:
        consts = ctx.enter_context(tc.tile_pool(name="consts", bufs=1))
        inp = ctx.enter_context(tc.tile_pool(name="inp", bufs=3))
        hlp = ctx.enter_context(tc.tile_pool(name="hlp", bufs=3))
        outp = ctx.enter_context(tc.tile_pool(name="outp", bufs=3))
        psum = ctx.enter_context(tc.tile_pool(name="psum", bufs=2, space="PSUM"))

        c32_sb = consts.tile([128, _CONST_COLS], cdt)
        nc.sync.dma_start(out=c32_sb[:], in_=c32[:])
        if MODE == "split":
            cbf_sb = consts.tile([128, 2 * _A_COLS], bf16)
            nc.sync.dma_start(out=cbf_sb[:], in_=cbf[:])

        ones_ap = c32_sb[0:1, _ONES_OFF : _ONES_OFF + W]

        for s in range(SPC):
            for G0, NG in CHUNKS:
                ti = inp.tile([128, 4, W], xdt, tag="ti")
                # ---- input DMAs, one per group (DMA APs max 3 dims)
                for g in range(NG):
                    G = G0 + g
                    if G == 0:
                        # zero the row(-1) slab: stale SBUF could hold NaNs
                        # and 0-weight x NaN still poisons the accumulation
                        nc.vector.memset(ti[0:32, 0:1, :], 0.0)
                        nc.sync.dma_start(
                            out=ti[8:128, 0:1, :],
                            in_=_dram_ap(xin, [[W, 15], [_HW, IC], [1, W]], s * _SAMP),
                        )
                    elif G == NGRP - 1:
                        nc.vector.memset(ti[64:128, g : g + 1, :], 0.0)
                        nc.sync.dma_start(
                            out=ti[0:72, g : g + 1, :],
                            in_=_dram_ap(
                                xin, [[W, 9], [_HW, IC], [1, W]], s * _SAMP + 503 * W
                            ),
                        )
                    else:
                        nc.sync.dma_start(
                            out=ti[:, g : g + 1, :],
                            in_=_dram_ap(
                                xin,
                                [[W, TIN], [_HW, IC], [1, W]],
                                s * _SAMP + (14 * G - 1) * W,
                            ),
                        )
                if MODE == "split":
                    th = hlp.tile([128, 4, W], bf16, tag="th")
                    tl = hlp.tile([128, 4, W], bf16, tag="tl")
                    for g in range(NG):
                        nc.vector.tensor_copy(
                            out=th[:, g : g + 1, :], in_=ti[:, g : g + 1, :]
                        )
                        nc.vector.tensor_sub(
                            out=tl[:, g : g + 1, :],
                            in0=ti[:, g : g + 1, :],
                            in1=th[:, g : g + 1, :],
                        )

                to = outp.tile([M_PART, 4 * W], f32, tag="to")
                ps = psum.tile([M_PART, 4 * W], f32)

                for g in range(NG):
                    G = G0 + g
                    v = 1 if G == 0 else (2 if G == NGRP - 1 else 0)
                    pcol = g * W
                    # bias pre-load: psum[:, :] = bias x ones  (start)
                    nc.tensor.matmul(
                        ps[:, pcol : pcol + W],
                        c32_sb[0:1, _BIAS_OFF + s * M_PART : _BIAS_OFF + (s + 1) * M_PART],
                        ones_ap,
                        start=True,
                        stop=False,
                        skip_group_check=True,
                    )
                    # dx taps: out cols [lo,hi) <- x cols [lo+dx-1, hi+dx-1)
                    taps = []
                    for dx in range(3):
                        lo = max(0, 1 - dx)
                        hi = W - max(0, dx - 1)
                        taps.append((dx, lo, hi))
                    if MODE == "split":
                        mm_ops = []
                        for dx, lo, hi in taps:
                            ah = cbf_sb[:, _a_col(s, v, dx) : _a_col(s, v, dx) + M_PART]
                            al = cbf_sb[
                                :,
                                _A_COLS + _a_col(s, v, dx) : _A_COLS
                                + _a_col(s, v, dx)
                                + M_PART,
                            ]
                            xh = th[:, g, lo + dx - 1 : hi + dx - 1]
                            xl = tl[:, g, lo + dx - 1 : hi + dx - 1]
                            mm_ops.append((ah, xh))
                            mm_ops.append((ah, xl))
                            mm_ops.append((al, xh))
                        for i, (a_ap, x_ap) in enumerate(mm_ops):
                            dx, lo, hi = taps[i // 3]
                            nc.tensor.matmul(
                                ps[:, pcol + lo : pcol + hi],
                                a_ap,
                                x_ap,
                                start=False,
                                stop=(i == len(mm_ops) - 1),
                                skip_group_check=True,
                            )
                    else:
                        for i, (dx, lo, hi) in enumerate(taps):
                            nc.tensor.matmul(
                                ps[:, pcol + lo : pcol + hi],
                                c32_sb[:, _a_col(s, v, dx) : _a_col(s, v, dx) + M_PART],
                                ti[:, g, lo + dx - 1 : hi + dx - 1],
                                start=False,
                                stop=(i == 2),
                                skip_group_check=True,
                            )

                # PSUM -> SBUF (single DVE copy per chunk), then DMA out
                nc.vector.tensor_copy(out=to[:, : NG * W], in_=ps[:, : NG * W])
                for g in range(NG):
                    G = G0 + g
                    if G == NGRP - 1:
                        nc.sync.dma_start(
                            out=_dram_ap(
                                yout,
                                [[W, 8], [_HW, OC], [1, W]],
                                s * OC * _HW + 504 * W,
                            ),
                            in_=to[0:64, g * W : (g + 1) * W],
                        )
                    else:
                        nc.sync.dma_start(
                            out=_dram_ap(
                                yout,
                                [[W, TOUT], [_HW, OC], [1, W]],
                                s * OC * _HW + 14 * G * W,
                            ),
                            in_=to[:, g * W : (g + 1) * W],
                        )


WP = W + 2  # host-padded row width (zero col at each edge; f32r needs even N)
_HWP = H * WP
_SAMP_P = IC * _HWP


def _emit_conv_pair(nc, tc, xin, yout, c32):
    """Row-pair layout: partition = (rq*8+ic) in [0,64), each partition's
    tile slice holds TWO consecutive (width-padded) image rows (~4KB
    contiguous DMA packets). 6 f32r tap matmuls (3 dx x 2 parity) + bias
    per group; x is host-padded to width 514 so every tap is N=512."""
    f32 = mybir.dt.float32
    f32r = mybir.dt.float32r
    import contextlib

    with contextlib.ExitStack() as ctx:
        consts = ctx.enter_context(tc.tile_pool(name="consts", bufs=1))
        inp = ctx.enter_context(tc.tile_pool(name="inp", bufs=4))
        outp = ctx.enter_context(tc.tile_pool(name="outp", bufs=3))
        psum = ctx.enter_context(tc.tile_pool(name="psum", bufs=2, space="PSUM"))

        c32_sb = consts.tile([128, _CONST_COLS], f32r)
        nc.sync.dma_start(out=c32_sb[:], in_=c32[:])
        ones_ap = c32_sb[0:1, _ONES_OFF : _ONES_OFF + W]

        for s in range(SPC):
            for G0, NG in CHUNKS:
                ti = inp.tile([64, 4, 2, WP], f32r, tag="ti")
                for g in range(NG):
                    G = G0 + g
                    if G == 0:
                        nc.vector.memset(ti[0:8, g, 0:1, :].bitcast(f32), 0.0)  # row -1
                        nc.sync.dma_start(
                            out=ti[0:8, g, 1:2, :],
                            in_=_dram_ap(xin, [[_HWP, IC], [1, WP]], s * _SAMP_P),
                        )
                        nc.sync.dma_start(
                            out=ti[8:64, g, :, :],
                            in_=_dram_ap(
                                xin,
                                [[2 * WP, 7], [_HWP, IC], [1, 2 * WP]],
                                s * _SAMP_P + WP,
                            ),
                        )
                    elif G == NGRP - 1:
                        nc.vector.memset(ti[32:64, g, :, :].bitcast(f32), 0.0)
                        nc.sync.dma_start(
                            out=ti[0:32, g, :, :],
                            in_=_dram_ap(
                                xin,
                                [[2 * WP, 4], [_HWP, IC], [1, 2 * WP]],
                                s * _SAMP_P + 503 * WP,
                            ),
                        )
                        nc.sync.dma_start(
                            out=ti[32:40, g, 0:1, :],
                            in_=_dram_ap(
                                xin, [[_HWP, IC], [1, WP]], s * _SAMP_P + 511 * WP
                            ),
                        )
                    else:
                        nc.sync.dma_start(
                            out=ti[:, g, :, :],
                            in_=_dram_ap(
                                xin,
                                [[2 * WP, 8], [_HWP, IC], [1, 2 * WP]],
                                s * _SAMP_P + (14 * G - 1) * WP,
                            ),
                        )

                to = outp.tile([M_PART, 4 * W], f32, tag="to")
                ps = psum.tile([M_PART, 4 * W], f32)

                for g in range(NG):
                    G = G0 + g
                    v = 1 if G == 0 else (2 if G == NGRP - 1 else 0)
                    pcol = g * W
                    nc.tensor.matmul(
                        ps[:, pcol : pcol + W],
                        c32_sb[0:1, _BIAS_OFF + s * M_PART : _BIAS_OFF + (s + 1) * M_PART],
                        ones_ap,
                        start=True,
                        stop=False,
                        skip_group_check=True,
                    )
                    for i, (dx, p) in enumerate(
                        [(dx, p) for dx in range(3) for p in range(2)]
                    ):
                        col = _a_col_pair(s, v, dx, p)
                        nc.tensor.matmul(
                            ps[:, pcol : pcol + W],
                            c32_sb[0:64, col : col + M_PART],
                            ti[:, g, p, dx : dx + W],
                            start=False,
                            stop=(i == 5),
                            skip_group_check=True,
                        )

                nc.vector.tensor_copy(out=to[:, : NG * W], in_=ps[:, : NG * W])
                for g in range(NG):
                    G = G0 + g
                    if G == NGRP - 1:
                        nc.gpsimd.dma_start(
                            out=_dram_ap(
                                yout, [[W, 8], [_HW, OC], [1, W]],
                                s * OC * _HW + 504 * W,
                            ),
                            in_=to[0:64, g * W : (g + 1) * W],
                        )
                    else:
                        nc.gpsimd.dma_start(
                            out=_dram_ap(
                                yout, [[W, TOUT], [_HW, OC], [1, W]],
                                s * OC * _HW + 14 * G * W,
                            ),
                            in_=to[:, g * W : (g + 1) * W],
                        )


def _emit_conv_f32r(nc, tc, xin, yout, c32):
    """K=128 (16 rows x 8 ic) layout, f32r taps: 3 matmuls per group,
    bias folded into the DVE PSUM->SBUF copy (tensor_scalar_add). x is
    host-padded to width 514 so every tap is N=512 (f32r needs even N)."""
    f32 = mybir.dt.float32
    f32r = mybir.dt.float32r
    import contextlib

    with contextlib.ExitStack() as ctx:
        consts = ctx.enter_context(tc.tile_pool(name="consts", bufs=1))
        inp = ctx.enter_context(tc.tile_pool(name="inp", bufs=6))
        outp = ctx.enter_context(tc.tile_pool(name="outp", bufs=4))
        psum = ctx.enter_context(tc.tile_pool(name="psum", bufs=2, space="PSUM"))

        c32_sb = consts.tile([128, _CONST_COLS], f32r)
        nc.sync.dma_start(out=c32_sb[:], in_=c32[:])
        bias_s = [
            c32_sb[0:M_PART, _BVEC_OFF + s : _BVEC_OFF + s + 1].bitcast(f32)
            for s in range(SPC)
        ]

        ci = 0  # alternate in-DMA issue between sync and scalar queues
        for s in range(SPC):
            for G0, NG in CHUNKS:
                ti = inp.tile([128, 4, WP], f32r, tag="ti")
                for g in range(NG):
                    G = G0 + g
                    eng = nc.sync if ci % 2 == 0 else nc.scalar
                    ci += 1
                    if G == 0:
                        nc.vector.memset(ti[0:32, g : g + 1, :].bitcast(f32), 0.0)
                        eng.dma_start(
                            out=ti[8:128, g : g + 1, :],
                            in_=_dram_ap(
                                xin, [[WP, 15], [_HWP, IC], [1, WP]], s * _SAMP_P
                            ),
                        )
                    elif G == NGRP - 1:
                        nc.vector.memset(ti[64:128, g : g + 1, :].bitcast(f32), 0.0)
                        eng.dma_start(
                            out=ti[0:72, g : g + 1, :],
                            in_=_dram_ap(
                                xin,
                                [[WP, 9], [_HWP, IC], [1, WP]],
                                s * _SAMP_P + 503 * WP,
                            ),
                        )
                    else:
                        eng.dma_start(
                            out=ti[:, g : g + 1, :],
                            in_=_dram_ap(
                                xin,
                                [[WP, TIN], [_HWP, IC], [1, WP]],
                                s * _SAMP_P + (14 * G - 1) * WP,
                            ),
                        )

                to = outp.tile([M_PART, 4 * W], f32, tag="to")
                ps = psum.tile([M_PART, 4 * W], f32)

                for g in range(NG):
                    G = G0 + g
                    v = 1 if G == 0 else (2 if G == NGRP - 1 else 0)
                    pcol = g * W
                    for dx in range(3):
                        col = _a_col(s, v, dx)
                        nc.tensor.matmul(
                            ps[:, pcol : pcol + W],
                            c32_sb[:, col : col + M_PART],
                            ti[:, g, dx : dx + W],
                            start=(dx == 0),
                            stop=(dx == 2),
                            skip_group_check=True,
                        )
                nc.vector.tensor_scalar_add(
                    out=to[:, : NG * W], in0=ps[:, : NG * W], scalar1=bias_s[s]
                )
                for g in range(NG):
                    G = G0 + g
                    pcol = g * W
                    if G == NGRP - 1:
                        nc.gpsimd.dma_start(
                            out=_dram_ap(
                                yout, [[W, 8], [_HW, OC], [1, W]],
                                s * OC * _HW + 504 * W,
                            ),
                            in_=to[0:64, pcol : pcol + W],
                        )
                    else:
                        nc.gpsimd.dma_start(
                            out=_dram_ap(
                                yout, [[W, TOUT], [_HW, OC], [1, W]],
                                s * OC * _HW + 14 * G * W,
                            ),
                            in_=to[:, pcol : pcol + W],
                        )


_NC_CACHE = {}


def _get_nc():
    if MODE in _NC_CACHE:
        return _NC_CACHE[MODE]
    f32 = mybir.dt.float32
    bf16 = mybir.dt.bfloat16
    nc = bass.Bass("TRN2", target_bir_lowering=False, debug=False, num_devices=NCORES)
    if MODE == "packed":
        xin = nc.declare_dram_parameter("x", [NJ * 128, WPK], bf16, isOutput=False)
        c_a = nc.declare_dram_parameter(
            "c_a",
            [128, SPC * 3 * M_PART],
            mybir.dt.float32r if PACKED_F32R else bf16,
            isOutput=False,
        )
        c_b = nc.declare_dram_parameter("c_b", [M_PART, SPC], f32, isOutput=False)
        yout = nc.declare_dram_parameter("y", [NJ * M_PART, W], bf16, isOutput=True)
        with TileContext(nc) as tc:
            _emit_conv_packed(nc, tc, xin, yout, c_a, c_b)
        _legalize_waits(nc)
        _NC_CACHE[MODE] = nc
        return nc
    cdt = mybir.dt.float32r if MODE in ("split", "f32r", "pair") else f32
    xdt = cdt if MODE in ("f32r", "f32", "pair") else f32
    xshape = [SPC, IC, H, WP] if MODE in ("pair", "f32r") else [SPC, IC, H, W]
    xin = nc.declare_dram_parameter("x", xshape, xdt, isOutput=False)
    c32 = nc.declare_dram_parameter("c32", [128, _CONST_COLS], cdt, isOutput=False)
    cbf = None
    if MODE == "split":
        cbf = nc.declare_dram_parameter(
            "cbf", [128, 2 * _A_COLS], mybir.dt.bfloat16, isOutput=False
        )
    yout = nc.declare_dram_parameter("y", [SPC, OC, H, W], f32, isOutput=True)
    with TileContext(nc) as tc:
        if MODE == "pair":
            _emit_conv_pair(nc, tc, xin, yout, c32)
        elif MODE == "f32r":
            _emit_conv_f32r(nc, tc, xin, yout, c32)
        else:
            _emit_conv(nc, tc, xin, yout, c32, cbf)
    _legalize_waits(nc)
    _NC_CACHE[MODE] = nc
    return nc


def _prepare(q, x, wm1, bm1, wm2, bm2, wt, bt, wb, bb):
    q, x = np.asarray(q, np.float32), np.asarray(x, np.float32)
    args = [np.asarray(a, np.float32) for a in (wm1, bm1, wm2, bm2, wt, bt, wb, bb)]
    w, b = _manifold(q, *args)
    if MODE == "packed":
        xr_cores = _pack_inputs_packed(x)
        in_maps = []
        for k in range(NCORES):
            sl = slice(k * SPC, (k + 1) * SPC)
            c_a, c_b = _build_consts_packed(w[sl], b[sl])
            in_maps.append(
                {"x": xr_cores[k].reshape(NJ * 128, WPK), "c_a": c_a, "c_b": c_b}
            )
        return _get_nc(), in_maps
    A = _build_stationaries_pair(w) if MODE == "pair" else _build_stationaries(w)

    if MODE in ("pair", "f32r"):
        xp = np.zeros((B, IC, H, WP), np.float32)
        xp[:, :, :, 1 : 1 + W] = x
        x = xp

    in_maps = []
    for k in range(NCORES):
        sl = slice(k * SPC, (k + 1) * SPC)
        C = _build_consts(A[sl], b[sl])
        im = {"x": np.ascontiguousarray(x[sl]), "c32": C}
        if MODE == "split":
            import ml_dtypes

            Ablock = C[:, :_A_COLS].astype(np.float32)
            Ah = Ablock.astype(ml_dtypes.bfloat16)
            Al = (Ablock - Ah.astype(np.float32)).astype(ml_dtypes.bfloat16)
            im["cbf"] = np.concatenate([Ah, Al], axis=1)
        in_maps.append(im)
    return _get_nc(), in_maps


def kernel(q, x, wm1, bm1, wm2, bm2, wt, bt, wb, bb):
    nc, in_maps = _prepare(q, x, wm1, bm1, wm2, bm2, wt, bt, wb, bb)
    br = run_bass_kernel_spmd(nc, in_maps, list(range(NCORES)))
    if MODE == "packed":
        return _unpack_output_packed(
            [np.asarray(r["y"]).reshape(-1) for r in br.results]
        )
    return np.concatenate([r["y"] for r in br.results], axis=0)

